# revision 1
# baseline (speedup 1.0000x reference)
"""AgentAttention TRN2 kernel.

Math (per batch b, head h):
  q,k,v = split_heads(x @ w_qkv.T)                    # (n, d) each, d=64
  qa  = softmax(q @ agent_h.T * scale, axis=m)        # (n, m), m=256
  ak  = softmax(agent_h @ k.T, axis=n)                # (m, n)
  kv  = softmax(ak @ v, axis=d)                       # (m, d)
  out = qa @ kv                                       # (n, d)

Softmax trick: softmax(X) @ Y == (exp(X) @ [Y, 1]) -> divide by last col.
All softmax reductions fold into matmul contractions via exp (no max
subtraction needed: |logits| < 60 << 88, f32 exp never overflows).

Sharding: 8 cores = 4 batches x 2 head-groups (4 heads each).

Dtypes: fp16 for the x/w/q/k/agent matmul path (10-bit mantissa ~ f32r
precision, but bf16-class PE speed + fast weight load); f32r for v/E2T
(E2T can exceed fp16 range: exp of logits up to ~59); bf16 for E1T/KV
(post-softmax values, errors damped by normalization).

Pass A streams x: fp16 cast, PE transpose, qk+v projections,
E2=exp(k@agT), kv accumulated per super-tile in PSUM then added into
SBUF, and E1T precomputed for heads 0-1 (spreads ACT exp load).  After
kv finalize, pass B computes E1T for heads 2-3 (from stored qT) and the
out-stage with row-normalization.
"""
import sys
import os

sys.path.insert(0, "/opt/trn_rl_repo")

import numpy as np

HEADS = 8
D = 64              # dim per head
M = 256             # agent tokens
DIM = 512
N = 8192            # sequence length
B = 4
SCALE = D ** -0.5
ST = 512            # pass-A token super-tile
NST = N // ST       # 16
SL = 1024           # pass-B token slice
NSL = N // SL       # 8

_cached = {}


def _build():
    import concourse.bass as bass
    import concourse.bacc as bacc
    import concourse.tile as tile
    from concourse import mybir, masks
    from contextlib import ExitStack

    f32 = mybir.dt.float32
    f32r = mybir.dt.float32r
    bf16 = mybir.dt.bfloat16
    fp16 = mybir.dt.float16
    EXP = mybir.ActivationFunctionType.Exp

    nc = bacc.Bacc("TRN2", target_bir_lowering=False, debug=False)

    x_ap = nc.dram_tensor("x", [N, DIM], f32, kind="ExternalInput").ap()
    wqk_ap = nc.dram_tensor("wqk", [DIM, 512], f32, kind="ExternalInput").ap()
    wv_ap = nc.dram_tensor("wv", [DIM, 256], f32, kind="ExternalInput").ap()
    ag_ap = nc.dram_tensor("ag", [128, 1024], f32, kind="ExternalInput").ap()
    out_ap = nc.dram_tensor("out", [N, 256], f32, kind="ExternalOutput").ap()

    with tile.TileContext(nc) as tc, ExitStack() as ctx:
        const = ctx.enter_context(tc.tile_pool(name="const", bufs=1))
        qtp = ctx.enter_context(tc.tile_pool(name="qtp", bufs=1))

        ident = const.tile([128, 128], f32, tag="ident")
        masks.make_identity(nc, ident[:])
        ident_h = const.tile([128, 128], fp16, tag="ident_h")
        masks.make_identity(nc, ident_h[:])

        with tc.tile_pool(name="stage", bufs=1) as stage:
            wqk_s = stage.tile([128, 4, 512], f32, tag="wqk_s")
            nc.sync.dma_start(wqk_s[:], wqk_ap.rearrange("(ci p) w -> p ci w", p=128))
            wqk_h = const.tile([128, 4, 512], fp16, tag="wqk_h")
            nc.vector.tensor_copy(wqk_h[:], wqk_s[:])

            wv_s = stage.tile([128, 4, 256], f32, tag="wv_s")
            nc.sync.dma_start(wv_s[:], wv_ap.rearrange("(ci p) w -> p ci w", p=128))
            wv_h = const.tile([128, 4, 256], fp16, tag="wv_h")
            nc.vector.tensor_copy(wv_h[:], wv_s[:])

            ag_s = stage.tile([128, 1024], f32, tag="ag_s")
            nc.sync.dma_start(ag_s[:], ag_ap[:])
            ag_h = const.tile([128, 1024], fp16, tag="ag_h")
            nc.vector.tensor_copy(ag_h[:], ag_s[:])

            ones_s = stage.tile([128, 4], f32, tag="ones_s")
            nc.gpsimd.memset(ones_s[:], 1.0)
            ones_r = const.tile([128, 4], f32r, tag="ones_r")
            nc.vector.tensor_copy(ones_r[:], ones_s[:])
            ones_b = const.tile([128, 4], fp16, tag="ones_b")
            nc.vector.tensor_copy(ones_b[:], ones_s[:])

        # persistent qT storage: one tile per head-pair, rows = [qA | qB]
        qT = [qtp.tile([128, N], fp16, tag=f"qT{hp}", name=f"qT{hp}")
              for hp in range(2)]

        # E1T precomputed for heads 0..1 over all n (pass A)
        e1pre = [[qtp.tile([128, N], fp16, tag=f"e1pre{j}_{mc}",
                           name=f"e1pre{j}_{mc}")
                  for mc in range(2)] for j in range(2)]

        # KV_aug per head per m-chunk: (128, 66) bf16
        kv_aug = [[const.tile([128, 66], fp16, tag=f"kva{j}_{mc}",
                              name=f"kva{j}_{mc}")
                   for mc in range(2)] for j in range(4)]

        # ================= PASS A =================
        with ExitStack() as actx:
            xp = actx.enter_context(tc.tile_pool(name="xp", bufs=2))
            xtp = actx.enter_context(tc.tile_pool(name="xtp", bufs=2))
            ktp = actx.enter_context(tc.tile_pool(name="ktp", bufs=3))
            e2tp = actx.enter_context(tc.tile_pool(name="e2tp", bufs=3))
            vp = actx.enter_context(tc.tile_pool(name="vp", bufs=5))
            pmm = actx.enter_context(tc.tile_pool(name="pmm", bufs=3, space="PSUM"))
            pmm2 = actx.enter_context(tc.tile_pool(name="pmm2", bufs=3, space="PSUM"))
            pkv = actx.enter_context(tc.tile_pool(name="pkv", bufs=1, space="PSUM"))
            ptr = actx.enter_context(tc.tile_pool(name="ptr", bufs=1, space="PSUM"))

            kv_sb = [const.tile([65, 256], f32, tag=f"kvsb{j}", name=f"kvsb{j}")
                     for j in range(4)]

            for st in range(NST):
                r0 = st * ST
                x_t = xp.tile([128, 4, DIM], f32, tag="x_t")
                nc.sync.dma_start(
                    x_t[:], x_ap[r0:r0 + ST, :].rearrange("(tt p) c -> p tt c", p=128))

                # cast to fp16, PE-transpose -> xT (128 c-part, 4 ci, 512 tok)
                x_h = xp.tile([128, 4, DIM], fp16, tag="x_h")
                nc.vector.tensor_copy(x_h[:], x_t[:])
                xT = xtp.tile([128, 4, ST], fp16, tag="xT")
                for ci in range(4):
                    tp = ptr.tile([128, 512], fp16, tag="pe_scr_h")
                    for tt in range(4):
                        nc.tensor.transpose(
                            tp[:, tt * 128:(tt + 1) * 128],
                            x_h[:, tt, ci * 128:(ci + 1) * 128],
                            ident_h[:])
                    nc.vector.tensor_copy(xT[:, ci, :], tp[:])

                # qk projection per head-pair: psum rows = [qA|qB] then [kA|kB]
                kT = {}
                for hp in range(2):
                    kT[hp] = ktp.tile([128, ST], fp16, tag="kT", name=f"kT{hp}")
                    for qk in range(2):
                        pq = pmm.tile([128, 512], f32, tag="pe_scr")
                        for ci in range(4):
                            nc.tensor.matmul(
                                pq[:],
                                wqk_h[:, ci, hp * 256 + qk * 128:
                                      hp * 256 + (qk + 1) * 128],
                                xT[:, ci, :],
                                start=(ci == 0), stop=(ci == 3))
                        if qk == 0:
                            nc.vector.tensor_copy(qT[hp][:, r0:r0 + ST], pq[:])
                        else:
                            nc.vector.tensor_copy(kT[hp][:], pq[:])

                # E1 precompute for heads 0,1 (uses qT written this st)
                for j in range(2):
                    hp, rb = j // 2, (j % 2) * 64
                    for mc in range(2):
                        pE = pmm.tile([128, 512], f32, tag="pe_scr",
                                      name=f"pE{st}_{j}_{mc}")
                        nc.tensor.matmul(
                            pE[:],
                            ag_h[rb:rb + 64,
                                 j * 256 + mc * 128:j * 256 + (mc + 1) * 128],
                            qT[hp][rb:rb + 64, r0:r0 + ST],
                            start=True, stop=True)
                        nc.scalar.activation(e1pre[j][mc][:, r0:r0 + ST],
                                             pE[:], EXP, scale=SCALE)

                # v projection per token-subtile: (128 tok, 4 heads*64)
                v_t = {}
                for tt in range(4):
                    pv = pmm2.tile([128, 512], f32, tag="pv")
                    for ci in range(4):
                        nc.tensor.matmul(
                            pv[:, 0:256],
                            xT[:, ci, tt * 128:(tt + 1) * 128],
                            wv_h[:, ci, :],
                            start=(ci == 0), stop=(ci == 3))
                    vt = vp.tile([128, 4, 65], f32r, tag="v_t")
                    nc.vector.tensor_copy(
                        vt[:, :, 0:64],
                        pv[:, 0:256].rearrange("p (j d) -> p j d", j=4))
                    nc.vector.tensor_copy(vt[:, :, 64], ones_r[:])
                    v_t[tt] = vt

                # E2 = exp(k @ agT) per head; kv accumulated per (st, head)
                kvp_st = {}
                for j in range(4):
                    hp, rb = j // 2, (j % 2) * 64
                    for half in range(2):
                        pe2 = pmm2.tile([128, 512], f32, tag="pv")
                        for s in range(2):
                            tt = half * 2 + s
                            nc.tensor.matmul(
                                pe2[:, s * 256:(s + 1) * 256],
                                kT[hp][rb:rb + 64, tt * 128:(tt + 1) * 128],
                                ag_h[rb:rb + 64, j * 256:(j + 1) * 256],
                                start=True, stop=True)
                        e2t = e2tp.tile([128, 512], f32r, tag="e2t")
                        nc.scalar.activation(e2t[:], pe2[:], EXP)
                        if half == 0:
                            kvp_st[j] = pkv.tile([65, 256], f32, tag="kvp",
                                                 name=f"kvp{st}_{j}")
                        for s in range(2):
                            tt = half * 2 + s
                            nc.tensor.matmul(
                                kvp_st[j][:],
                                v_t[tt][:, j, :],
                                e2t[:, s * 256:(s + 1) * 256],
                                start=(tt == 0),
                                stop=(tt == 3))
                    if st == 0:
                        nc.vector.tensor_copy(kv_sb[j][:], kvp_st[j][:])
                    else:
                        nc.vector.tensor_tensor(kv_sb[j][:], kv_sb[j][:],
                                                kvp_st[j][:],
                                                mybir.AluOpType.add)

            # ---- kv finalize per head ----
            fin = actx.enter_context(tc.tile_pool(name="fin", bufs=1))
            for j in range(4):
                kvs = kv_sb[j]
                for mc in range(2):
                    pt = pmm.tile([128, 512], f32, tag="pe_scr")
                    nc.tensor.transpose(
                        pt[:, 0:65], kvs[:, mc * 128:(mc + 1) * 128],
                        ident[0:65, 0:65])
                    den = fin.tile([128, 1], f32, tag=f"den{j}{mc}")
                    nc.vector.reciprocal(den[:], pt[:, 64:65])
                    kve = fin.tile([128, 64], f32, tag=f"kve{j}{mc}")
                    esum = fin.tile([128, 1], f32, tag=f"es{j}{mc}")
                    nc.scalar.activation(kve[:], pt[:, 0:64], EXP,
                                         scale=den[:], accum_out=esum[:])
                    rsum = fin.tile([128, 1], f32, tag=f"rs{j}{mc}")
                    nc.vector.reciprocal(rsum[:], esum[:])
                    nc.vector.tensor_scalar_mul(kv_aug[j][mc][:, 0:64],
                                                kve[:], rsum[:])
                    nc.vector.tensor_copy(kv_aug[j][mc][:, 64:66],
                                          ones_b[:, 0:2])

        # ================= PASS B =================
        with ExitStack() as bctx:
            e1tp = bctx.enter_context(tc.tile_pool(name="e1tp", bufs=10))
            outp = bctx.enter_context(tc.tile_pool(name="outp", bufs=4))
            pe1 = bctx.enter_context(tc.tile_pool(name="pe1", bufs=2, space="PSUM"))
            pout = bctx.enter_context(tc.tile_pool(name="pout", bufs=4, space="PSUM"))

            for sl in range(NSL):
                c0 = sl * SL
                e1t = {}
                for j in range(2, 4):
                    hp, rb = j // 2, (j % 2) * 64
                    for mc in range(2):
                        pp = pe1.tile([128, SL], f32, tag="pe1")
                        for half in range(2):
                            nc.tensor.matmul(
                                pp[:, half * 512:(half + 1) * 512],
                                ag_h[rb:rb + 64,
                                     j * 256 + mc * 128:j * 256 + (mc + 1) * 128],
                                qT[hp][rb:rb + 64,
                                       c0 + half * 512:c0 + (half + 1) * 512],
                                start=True, stop=True)
                        t = e1tp.tile([128, SL], fp16, tag="e1t")
                        nc.scalar.activation(t[:], pp[:], EXP, scale=SCALE)
                        e1t[(j, mc)] = t

                for tt in range(8):
                    po = pout.tile([128, 4, 66], f32, tag="pout")
                    for j in range(4):
                        for mc in range(2):
                            src = (e1pre[j][mc][:, c0 + tt * 128:
                                                c0 + (tt + 1) * 128]
                                   if j < 2 else
                                   e1t[(j, mc)][:, tt * 128:(tt + 1) * 128])
                            nc.tensor.matmul(
                                po[:, j, :],
                                src,
                                kv_aug[j][mc][:],
                                start=(mc == 0), stop=(mc == 1))
                    rec = outp.tile([128, 4], f32, tag="rec")
                    nc.vector.reciprocal(rec[:], po[:, :, 64])
                    ot = outp.tile([128, 4, 64], f32, tag="ot")
                    nc.vector.tensor_tensor(
                        ot[:], po[:, :, 0:64],
                        rec[:].unsqueeze(2).broadcast_to((128, 4, 64)),
                        mybir.AluOpType.mult)
                    nc.sync.dma_start(
                        out_ap[c0 + tt * 128:c0 + (tt + 1) * 128, :],
                        ot[:].rearrange("p j d -> p (j d)"))

    nc.compile()
    return nc


def _get_program():
    if "nc" not in _cached:
        _cached["nc"] = _build()
    return _cached["nc"]


def kernel(x, w_qkv, agent):
    from concourse.bass_utils import run_bass_kernel_spmd

    nc = _get_program()

    x = np.ascontiguousarray(x, dtype=np.float32)
    w_qkv = np.asarray(w_qkv, dtype=np.float32)
    agent = np.asarray(agent, dtype=np.float32)

    in_maps = []
    for core in range(8):
        bi, hg = core // 2, core % 2
        heads = [4 * hg + jj for jj in range(4)]
        wqk = np.empty((DIM, 512), np.float32)
        for hp in range(2):
            hA, hB = heads[2 * hp], heads[2 * hp + 1]
            wqk[:, hp * 256 + 0:hp * 256 + 64] = w_qkv[hA * 64:(hA + 1) * 64, :].T
            wqk[:, hp * 256 + 64:hp * 256 + 128] = w_qkv[hB * 64:(hB + 1) * 64, :].T
            wqk[:, hp * 256 + 128:hp * 256 + 192] = \
                w_qkv[DIM + hA * 64:DIM + (hA + 1) * 64, :].T
            wqk[:, hp * 256 + 192:hp * 256 + 256] = \
                w_qkv[DIM + hB * 64:DIM + (hB + 1) * 64, :].T
        wv = np.empty((DIM, 256), np.float32)
        for jj, hh in enumerate(heads):
            wv[:, jj * 64:(jj + 1) * 64] = \
                w_qkv[2 * DIM + hh * 64:2 * DIM + (hh + 1) * 64, :].T
        ag = np.empty((128, 1024), np.float32)
        for jj, hh in enumerate(heads):
            agT = agent[hh].T
            ag[0:64, jj * 256:(jj + 1) * 256] = agT
            ag[64:128, jj * 256:(jj + 1) * 256] = agT
        in_maps.append({"x": x[bi], "wqk": wqk, "wv": wv, "ag": ag})

    res = run_bass_kernel_spmd(nc, in_maps, core_ids=list(range(8)),
                               trace=bool(os.environ.get("AGENT_TRACE")))
    out = np.empty((B, N, DIM), np.float32)
    for core in range(8):
        bi, hg = core // 2, core % 2
        out[bi, :, hg * 256:(hg + 1) * 256] = res.results[core]["out"]
    if res.exec_time_ns is not None:
        kernel.last_exec_time_ns = res.exec_time_ns
        kernel.last_mean_exec_time_ns = res.mean_exec_time_ns
        kernel.last_trace = res.instructions_and_trace
    return out

